# revision 32
# baseline (speedup 1.0000x reference)
"""Gemma3 sliding-window attention on 8 Trainium2 NeuronCores.

Sharding: tensor-parallel over the 8 query heads (1 head per core). Each core
computes q/k/v projections for its head (KV head replicated per pair of
cores), RMS norm + RoPE, windowed-causal softcapped attention, and its o_proj
partial; the host sums the 8 partials.

Kernel layout notes:
- All matmuls run in bf16 (fp32 PSUM accumulation).
- Scores are computed transposed (sT[j, q] strips, j on partitions) so the
  exp output is directly the PV matmul's moving-side input - no PE
  transposes anywhere.
- Softcap tanh bounds logits to +-SOFTCAP, so softmax runs without the
  max-subtraction pass (exp(+-50) is safe in fp32).
- (1 + rms_w) is folded into Wq/Wk on the host; rstd_q is applied to q via a
  DRAM-broadcast row; rstd_k is folded into the tanh activation scale;
  1/denominator is folded into the o_proj PSUM-evacuation scale.
"""

import numpy as np
import ml_dtypes

import concourse.bass as bass
from concourse import mybir
from concourse.bass_utils import run_bass_kernel_spmd
from concourse.tile import TileContext

B, S, HID = 1, 2048, 2560
H, KV, D = 8, 4, 256
SCALE = 256 ** -0.5
SOFTCAP = 50.0
WINDOW = 512
EPS = 1e-6

N_CORES = 8
BF16 = mybir.dt.bfloat16
F32 = mybir.dt.float32
KT = HID // 128          # 20 contraction tiles for projections
NCH = S // 512           # 4 query chunks of 512
NSB = S // 128           # 16 seq blocks of 128
STRIPW = WINDOW + 128    # 640: score strip width per j-block
SH = S // 2              # per-core K/V sequence half
PAIRS = [[0, 1], [2, 3], [4, 5], [6, 7]]


def _split_multi_waits(nc):
    """This walrus build accepts only ONE sync-wait per instruction, but Tile
    attaches one wait per cross-proc dependency. Hoist extra waits onto
    freshly inserted same-engine NOPs directly before the instruction."""
    import bass_rust as _br

    uid = 0
    for f in nc.m.functions:
        for blk in f.blocks:
            il = blk.instructions
            new = []
            changed = False
            for inst in il:
                si = inst.sync_info
                if si is not None and len(si.on_wait) > 1:
                    conds = list(si.on_wait)
                    for c in conds[:-1]:
                        uid += 1
                        nop = mybir.InstNoOp(
                            name=f"splitw-{uid}",
                            engine=inst.engine,
                            sync_info=mybir.SyncInfo(on_wait=[c], on_update=[]),
                            bass_nofuse=True,
                        )
                        new.append(nop)
                    inst.sync_info = _br.SyncInfo(
                        on_wait=[conds[-1]], on_update=list(si.on_update)
                    )
                    changed = True
                new.append(inst)
            if changed:
                blk.instructions = new


def _build_program():
    nc = bass.Bass(num_devices=N_CORES)

    hsT = nc.dram_tensor("hsT", [HID, S], BF16, kind="ExternalInput")
    wqkvT = nc.dram_tensor("wqkvT", [HID, 3 * D], BF16, kind="ExternalInput")
    woT = nc.dram_tensor("woT", [D, HID], BF16, kind="ExternalInput")
    cosT = nc.dram_tensor("cosT", [D, S], BF16, kind="ExternalInput")
    sinT = nc.dram_tensor("sinT", [D, S], BF16, kind="ExternalInput")
    cq = nc.dram_tensor("cq", [D, 1], F32, kind="ExternalInput")
    ck = nc.dram_tensor("ck", [D, 1], F32, kind="ExternalInput")
    mask01 = nc.dram_tensor("mask01", [128, STRIPW], BF16, kind="ExternalInput")
    o_partial = nc.dram_tensor("o_partial", [S, HID], BF16, kind="ExternalOutput")
    # DRAM scratch rows used to broadcast / transpose per-token scalars.
    rq_row_d = nc.dram_tensor("rq_row_d", [1, S], F32)
    rk_row_d = nc.dram_tensor("rk_row_d", [1, S], F32)
    rd_row_d = nc.dram_tensor("rd_row_d", [1, S], F32)

    with TileContext(nc) as tc:
        _emit(nc, tc, hsT, wqkvT, woT, cosT, sinT, cq, ck, mask01,
              o_partial, rq_row_d, rk_row_d, rd_row_d)
    _split_multi_waits(nc)
    return nc


def _emit(nc, tc, hsT, wqkvT, woT, cosT, sinT, cq, ck, mask01,
          o_partial, rq_row_d, rk_row_d, rd_row_d):
    with (
        tc.tile_pool(name="persist", bufs=1) as persist,
        tc.tile_pool(name="rows", bufs=2) as row_pool,
        tc.tile_pool(name="t", bufs=2) as t_pool,
        tc.tile_pool(name="sc_ps", bufs=2, space="PSUM") as sc_ps,
    ):
        # Persistent SBUF tensors, split per 512-chunk to avoid false deps.
        qn_c = [[persist.tile([128, 512], BF16, tag=f"qn{c}{m}", name="t_qn")
                 for m in range(2)] for c in range(NCH)]
        kn_c = [[persist.tile([128, 512], BF16, tag=f"kn{c}{m}", name="t_kn")
                 for m in range(2)] for c in range(NCH)]
        v_c = [persist.tile([128, 4, D], BF16, tag=f"v{c}", name="t_v")
               for c in range(NCH)]
        ao_c = [[persist.tile([128, 512], BF16, tag=f"ao{c}{m}", name="t_ao")
                 for m in range(2)] for c in range(NCH)]
        wo_sb = persist.tile([128, 2, HID], BF16, tag="wo", name="t_wo")
        mask_sb = persist.tile([128, STRIPW], BF16, tag="mask", name="t_mask")
        cq_sb = persist.tile([128, 2], F32, tag="cq", name="t_cq")
        ck_sb = persist.tile([128, 2], F32, tag="ck", name="t_ck")
        ones_sb = persist.tile([128, 1], BF16, tag="ones", name="t_ones")
        rkT_c = [persist.tile([128, 4], F32, tag=f"rkT{c}", name="t_rkT")
                 for c in range(NCH)]
        rdT_c = [persist.tile([128, 4], F32, tag=f"rdT{c}", name="t_rdT")
                 for c in range(NCH)]
        eps_sb = persist.tile([1, 1], F32, tag="eps", name="t_eps")

        nc.gpsimd.dma_start(out=wo_sb,
                            in_=woT.ap().rearrange("(m p) c -> p m c", p=128))
        nc.gpsimd.dma_start(out=mask_sb, in_=mask01.ap())
        nc.gpsimd.dma_start(out=cq_sb,
                            in_=cq.ap().rearrange("(m p) one -> p (m one)", p=128))
        nc.gpsimd.dma_start(out=ck_sb,
                            in_=ck.ap().rearrange("(m p) one -> p (m one)", p=128))
        nc.vector.memset(ones_sb, 1.0)
        nc.vector.memset(eps_sb, EPS)

        tts = [None] * NSB

        def emit_strip_tanh(jb):
            q0 = jb * 128
            cb = jb // 4
            lb = jb % 4
            w = min(STRIPW, S - q0)
            tt = t_pool.tile([128, STRIPW], BF16, tag="tt", name="t_tt",
                             bufs=9)
            b0 = (4 - lb) * 128                      # cols until chunk boundary
            parts = [(0, min(b0, w))]
            if w > b0:
                parts.append((b0, w - b0))
            for off, n in parts:
                ci = cb + (1 if off >= b0 else 0)
                lo = q0 + off - ci * 512             # offset within chunk tile
                ps = sc_ps.tile([128, 512], F32, tag="sc", name="t_sc")
                for m in range(2):
                    nc.tensor.matmul(
                        ps[:, :n], kn_c[cb][m][:, lb * 128:(lb + 1) * 128],
                        qn_c[ci][m][:, lo:lo + n],
                        start=(m == 0), stop=(m == 1))
                nc.scalar.activation(tt[:, off:off + n], ps[:, :n],
                                     mybir.ActivationFunctionType.Tanh,
                                     scale=rkT_c[cb][:, lb:lb + 1])
            tts[jb] = tt

        # ---- Phase B/C: projections + rmsnorm + rope over 4 s-chunks ----
        with (
            tc.tile_pool(name="wpool", bufs=1) as wpool,
            tc.tile_pool(name="hs", bufs=42) as hs_pool,
            tc.tile_pool(name="cs", bufs=2) as cs_pool,
            tc.tile_pool(name="ep", bufs=2) as ep_pool,
            tc.tile_pool(name="qk_ps", bufs=1, space="PSUM") as qk_ps,
            tc.tile_pool(name="v_ps", bufs=2, space="PSUM") as v_ps,
        ):
            wqkv_k = [wpool.tile([128, 3 * D], BF16, tag=f"wqkv{k}",
                                 name="t_wqkv") for k in range(KT)]

            def emit_chunk_b(c, hs_list, off, cos_p, sin_p):
                s0 = c * 512
                qp = [qk_ps.tile([128, 512], F32, tag=f"qp{m}", name="t_qp")
                      for m in range(2)]
                kp = [qk_ps.tile([128, 512], F32, tag=f"kp{m}", name="t_kp")
                      for m in range(2)]
                vp01 = [v_ps.tile([128, D], F32, tag="vp", name="t_vp")
                        for _ in range(2)]
                for k in range(KT):
                    hs_t = hs_list[k]
                    st, sp = (k == 0), (k == KT - 1)
                    for m in range(2):
                        nc.tensor.matmul(qp[m],
                                         wqkv_k[k][:, m * 128:(m + 1) * 128],
                                         hs_t[:, off:off + 512],
                                         start=st, stop=sp)
                    for m in range(2):
                        nc.tensor.matmul(kp[m],
                                         wqkv_k[k][:, D + m * 128:D + (m + 1) * 128],
                                         hs_t[:, off:off + 512],
                                         start=st, stop=sp)
                    for sb in range(2):
                        nc.tensor.matmul(vp01[sb],
                                         hs_t[:, off + sb * 128:off + (sb + 1) * 128],
                                         wqkv_k[k][:, 2 * D:3 * D],
                                         start=st, stop=sp)
                for sb in range(2):
                    nc.vector.tensor_copy(v_c[c][:, sb, :], vp01[sb])
                vp23 = [v_ps.tile([128, D], F32, tag="vp", name="t_vp")
                        for _ in range(2)]
                for k in range(KT):
                    st, sp = (k == 0), (k == KT - 1)
                    for sb in (2, 3):
                        nc.tensor.matmul(vp23[sb - 2],
                                         hs_list[k][:, off + sb * 128:off + (sb + 1) * 128],
                                         wqkv_k[k][:, 2 * D:3 * D],
                                         start=st, stop=sp)
                for sb in (2, 3):
                    nc.vector.tensor_copy(v_c[c][:, sb, :], vp23[sb - 2])

                # ACT scaled squares (c_d in scale), DVE evacuation copies
                sqs = {}
                for which, pp, cvec in (("q", qp, cq_sb), ("k", kp, ck_sb)):
                    for m in range(2):
                        sq = ep_pool.tile([128, 512], BF16, bufs=1,
                                          tag=f"sq_{which}{m}", name="t_sq")
                        nc.scalar.activation(sq, pp[m],
                                             mybir.ActivationFunctionType.Square,
                                             scale=cvec[:, m:m + 1])
                        sqs[which, m] = sq
                qt = []
                for m in range(2):
                    qt_m = ep_pool.tile([128, 512], BF16, tag=f"qt{m}", name="t_qt")
                    nc.vector.tensor_copy(qt_m, qp[m])
                    qt.append(qt_m)
                kh = []
                for m in range(2):
                    kh_m = ep_pool.tile([128, 512], BF16, tag=f"kh{m}", name="t_kh")
                    nc.vector.tensor_copy(kh_m, kp[m])
                    kh.append(kh_m)

                rows = {}
                for which in ("q", "k"):
                    msq = v_ps.tile([1, 512], F32, tag="vp", name="t_msq")
                    for m in range(2):
                        nc.tensor.matmul(msq, ones_sb, sqs[which, m],
                                         start=(m == 0), stop=(m == 1))
                    rows[which] = msq
                srts = {}
                for which in ("q", "k"):
                    srt = row_pool.tile([1, 512], F32, tag=f"srt_{which}",
                                        name="t_srt")
                    nc.scalar.activation(srt, rows[which],
                                         mybir.ActivationFunctionType.Sqrt,
                                         bias=eps_sb)
                    srts[which] = srt
                rrow_q = row_pool.tile([1, 512], F32, tag="rrow_q", name="t_rrow")
                nc.vector.reciprocal(rrow_q, srts["q"])
                nc.gpsimd.dma_start(out=rq_row_d[0:1, s0:s0 + 512], in_=rrow_q)
                rrow_k = row_pool.tile([1, 512], F32, tag="rrow_k", name="t_rrow")
                nc.vector.reciprocal(rrow_k, srts["k"])
                rrow2 = row_pool.tile([1, 512], F32, tag="rrow2", name="t_rrow2")
                nc.vector.tensor_scalar_mul(rrow2, rrow_k, SCALE / SOFTCAP)
                nc.gpsimd.dma_start(out=rk_row_d[0:1, s0:s0 + 512], in_=rrow2)

                # broadcast rstd_q across partitions; transpose rstd_k to cols
                rq_bc = ep_pool.tile([128, 512], F32, bufs=1, tag="rq_bc", name="t_rq_bc")
                nc.gpsimd.dma_start(
                    out=rq_bc,
                    in_=bass.AP(tensor=rq_row_d, offset=s0,
                                ap=[[0, 128], [1, 512]]))
                nc.gpsimd.dma_start(
                    out=rkT_c[c],
                    in_=bass.AP(tensor=rk_row_d, offset=s0,
                                ap=[[1, 128], [128, 4]]))

                qh = []
                for m in range(2):
                    qh_m = ep_pool.tile([128, 512], BF16, tag=f"qh{m}", name="t_qh")
                    nc.vector.tensor_mul(qh_m, qt[m], rq_bc)
                    qh.append(qh_m)
                for src_t, dst in ((qh, qn_c[c]), (kh, kn_c[c])):
                    for m in range(2):
                        t0 = ep_pool.tile([128, 512], BF16, tag="rope_t0",
                                          name="t_rope_t0")
                        t1 = ep_pool.tile([128, 512], BF16, tag="rope_t1",
                                          name="t_rope_t1")
                        nc.vector.tensor_mul(t0, src_t[m],
                                             cos_p[m][:, off:off + 512])
                        nc.vector.tensor_mul(t1, src_t[1 - m],
                                             sin_p[m][:, off:off + 512])
                        if m == 0:
                            nc.vector.tensor_sub(dst[m], t0, t1)
                        else:
                            nc.vector.tensor_add(dst[m], t0, t1)

            for c in range(NCH):
                s0 = c * 512
                hs_list = []
                for k in range(KT):
                    hs_t = hs_pool.tile([128, 512], BF16, tag="hs", name="t_hs")
                    nc.sync.dma_start(
                        out=hs_t,
                        in_=hsT[k * 128:(k + 1) * 128, s0:s0 + 512])
                    hs_list.append(hs_t)
                    if c == 0:
                        nc.sync.dma_start(
                            out=wqkv_k[k],
                            in_=wqkvT[k * 128:(k + 1) * 128, :])
                cos_p = [cs_pool.tile([128, 512], BF16, tag=f"cos{m}",
                                      name="t_cos") for m in range(2)]
                sin_p = [cs_pool.tile([128, 512], BF16, tag=f"sin{m}",
                                      name="t_sin") for m in range(2)]
                for m in range(2):
                    nc.gpsimd.dma_start(
                        out=cos_p[m],
                        in_=cosT[m * 128:(m + 1) * 128, s0:s0 + 512])
                    nc.gpsimd.dma_start(
                        out=sin_p[m],
                        in_=sinT[m * 128:(m + 1) * 128, s0:s0 + 512])
                emit_chunk_b(c, hs_list, 0, cos_p, sin_p)

        # ---- Phases D/E/F/G: score strips, softmax, PV, o_proj ----
        with (
            tc.tile_pool(name="e", bufs=9) as e_pool,
            tc.tile_pool(name="oc", bufs=2) as oc_pool,
            tc.tile_pool(name="pv_ps", bufs=2, space="PSUM") as pv_ps,
            tc.tile_pool(name="dn_ps", bufs=1, space="PSUM") as dn_ps,
            tc.tile_pool(name="og_ps", bufs=3, space="PSUM") as og_ps,
        ):
            e_tiles = [None] * NSB

            def emit_strip_exp(jb):
                q0 = jb * 128
                w = min(STRIPW, S - q0)
                er = t_pool.tile([128, STRIPW], BF16, tag="er", name="t_er")
                nc.scalar.activation(er[:, :w], tts[jb][:, :w],
                                     mybir.ActivationFunctionType.Exp,
                                     scale=SOFTCAP)
                e = e_pool.tile([128, STRIPW], BF16, tag="e", name="t_e")
                nc.vector.tensor_mul(e[:, :w], er[:, :w], mask_sb[:, :w])
                e_tiles[jb] = e

            def emit_chunk(c):
                q0 = c * 512
                jbs = [jb for jb in range(max(0, 4 * c - 4), 4 * c + 4)
                       if jb * 128 + STRIPW > q0]
                ops = [pv_ps.tile([128, 512], F32, tag="pv", name="t_pv")
                       for _ in range(2)]
                dn = dn_ps.tile([1, 512], F32, tag="dn", name="t_dn")
                for i, jb in enumerate(jbs):
                    sw = min(STRIPW, S - jb * 128)
                    lo = max(q0, jb * 128)
                    hi = min(q0 + 512, jb * 128 + sw)
                    el = lo - jb * 128
                    co = lo - q0
                    n = hi - lo
                    st, sp = (i == 0), (i == len(jbs) - 1)
                    e = e_tiles[jb]
                    for m in range(2):
                        nc.tensor.matmul(ops[m][:, co:co + n],
                                         v_c[jb // 4][:, jb % 4,
                                                      m * 128:(m + 1) * 128],
                                         e[:, el:el + n], start=st, stop=sp,
                                         skip_group_check=True)
                    nc.tensor.matmul(dn[:, co:co + n], ones_sb,
                                     e[:, el:el + n], start=st, stop=sp,
                                     skip_group_check=True)
                rd = row_pool.tile([1, 512], F32, tag="rd", name="t_rd")
                nc.vector.reciprocal(rd, dn)
                nc.sync.dma_start(out=rd_row_d[0:1, q0:q0 + 512], in_=rd)
                nc.sync.dma_start(
                    out=rdT_c[c],
                    in_=bass.AP(tensor=rd_row_d, offset=q0,
                                ap=[[1, 128], [128, 4]]))
                for m in range(2):
                    nc.vector.tensor_copy(ao_c[c][m], ops[m])

            def emit_oproj(sb):
                oc = oc_pool.tile([128, HID], BF16, tag="oc", name="t_oc")
                for cc in range(HID // 512):
                    op = og_ps.tile([128, 512], F32, tag="og", name="t_og")
                    for m in range(2):
                        nc.tensor.matmul(op,
                                         ao_c[sb // 4][m][:, (sb % 4) * 128:
                                                          (sb % 4 + 1) * 128],
                                         wo_sb[:, m, cc * 512:(cc + 1) * 512],
                                         start=(m == 0), stop=(m == 1))
                    if cc % 2 == 0:
                        nc.vector.tensor_scalar_mul(
                            oc[:, cc * 512:(cc + 1) * 512], op,
                            rdT_c[sb // 4][:, sb % 4:sb % 4 + 1])
                    else:
                        nc.scalar.activation(
                            oc[:, cc * 512:(cc + 1) * 512], op,
                            mybir.ActivationFunctionType.Copy,
                            scale=rdT_c[sb // 4][:, sb % 4:sb % 4 + 1])
                nc.sync.dma_start(
                    out=o_partial[sb * 128:(sb + 1) * 128, :], in_=oc)

            # Interleave strips -> chunk PV -> this chunk's o_proj.
            for c in range(NCH):
                for jb in range(4 * c, 4 * c + 4):
                    emit_strip_tanh(jb)
                for jb in range(4 * c, 4 * c + 4):
                    emit_strip_exp(jb)
                emit_chunk(c)
                for sb in range(4 * c, 4 * c + 4):
                    emit_oproj(sb)


_PROGRAM = None


def _get_program():
    global _PROGRAM
    if _PROGRAM is None:
        _PROGRAM = _build_program()
    return _PROGRAM


def _prep_inputs(hidden_states, position_ids, cos_table, sin_table,
                 Wq, Wk, Wv, Wo, q_norm_w, k_norm_w):
    bf16 = ml_dtypes.bfloat16
    f32 = np.float32
    hs = np.asarray(hidden_states, dtype=f32).reshape(S, HID)
    hsT = np.ascontiguousarray(hs.T).astype(bf16)
    pos = np.asarray(position_ids).reshape(S).astype(np.int64)
    cosT = np.ascontiguousarray(
        np.asarray(cos_table, dtype=f32)[pos].T).astype(bf16)
    sinT = np.ascontiguousarray(
        np.asarray(sin_table, dtype=f32)[pos].T).astype(bf16)
    Wq = np.asarray(Wq, dtype=f32); Wk = np.asarray(Wk, dtype=f32)
    Wv = np.asarray(Wv, dtype=f32); Wo = np.asarray(Wo, dtype=f32)
    qw = 1.0 + np.asarray(q_norm_w, dtype=f32)
    kw = 1.0 + np.asarray(k_norm_w, dtype=f32)
    cq = (np.abs(qw) ** -1 / np.sqrt(D)).astype(f32).reshape(D, 1)
    ck = (np.abs(kw) ** -1 / np.sqrt(D)).astype(f32).reshape(D, 1)
    p = np.arange(128)[:, None]
    f = np.arange(STRIPW)[None, :]
    mask01 = (((f - p) >= 0) & ((f - p) < WINDOW)).astype(bf16)

    in_maps = []
    for h in range(N_CORES):
        kv = h // (H // KV)
        wq_s = Wq[h * D:(h + 1) * D, :] * qw[:, None]
        wk_s = Wk[kv * D:(kv + 1) * D, :] * kw[:, None]
        wv_s = Wv[kv * D:(kv + 1) * D, :]
        wo_s = Wo[:, h * D:(h + 1) * D]
        wqkv = np.concatenate([wq_s.T, wk_s.T, wv_s.T], axis=1)
        in_maps.append({
            "hsT": hsT,
            "wqkvT": np.ascontiguousarray(wqkv).astype(bf16),
            "woT": np.ascontiguousarray(wo_s.T).astype(bf16),
            "cosT": cosT, "sinT": sinT,
            "cq": cq, "ck": ck, "mask01": mask01,
        })
    return in_maps


def _run(in_maps, trace=False):
    nc = _get_program()
    return run_bass_kernel_spmd(nc, in_maps, list(range(N_CORES)), trace=trace)


def kernel(hidden_states, position_ids, cos_table, sin_table,
           Wq, Wk, Wv, Wo, q_norm_w, k_norm_w):
    in_maps = _prep_inputs(hidden_states, position_ids, cos_table, sin_table,
                           Wq, Wk, Wv, Wo, q_norm_w, k_norm_w)
    res = _run(in_maps)
    out = np.zeros((S, HID), dtype=np.float32)
    for r in res.results:
        out += r["o_partial"].astype(np.float32)
    return out.reshape(B, S, HID)


# revision 34
# speedup vs baseline: 1.1299x; 1.1299x over previous
"""Gemma3 sliding-window attention on 8 Trainium2 NeuronCores.

Sharding: tensor-parallel over the 8 query heads (1 head per core). Each core
computes q/k/v projections for its head (KV head replicated per pair of
cores), RMS norm + RoPE, windowed-causal softcapped attention, and its o_proj
partial; the host sums the 8 partials.

Kernel layout notes:
- All matmuls run in bf16 (fp32 PSUM accumulation).
- Scores are computed transposed (sT[j, q] strips, j on partitions) so the
  exp output is directly the PV matmul's moving-side input - no PE
  transposes anywhere.
- Softcap tanh bounds logits to +-SOFTCAP, so softmax runs without the
  max-subtraction pass (exp(+-50) is safe in fp32).
- (1 + rms_w) is folded into Wq/Wk on the host; rstd_q is applied to q via a
  DRAM-broadcast row; rstd_k is folded into the tanh activation scale;
  1/denominator is folded into the o_proj PSUM-evacuation scale.
"""

import numpy as np
import ml_dtypes

import concourse.bass as bass
from concourse import mybir
from concourse.bass_utils import run_bass_kernel_spmd
from concourse.tile import TileContext

B, S, HID = 1, 2048, 2560
H, KV, D = 8, 4, 256
SCALE = 256 ** -0.5
SOFTCAP = 50.0
WINDOW = 512
EPS = 1e-6

N_CORES = 8
BF16 = mybir.dt.bfloat16
F32 = mybir.dt.float32
KT = HID // 128          # 20 contraction tiles for projections
NCH = S // 512           # 4 query chunks of 512
NSB = S // 128           # 16 seq blocks of 128
STRIPW = WINDOW + 128    # 640: score strip width per j-block
SH = S // 2              # per-core K/V sequence half
PAIRS = [[0, 1], [2, 3], [4, 5], [6, 7]]


def _split_multi_waits(nc):
    """This walrus build accepts only ONE sync-wait per instruction, but Tile
    attaches one wait per cross-proc dependency. Hoist extra waits onto
    freshly inserted same-engine NOPs directly before the instruction."""
    import bass_rust as _br

    uid = 0
    for f in nc.m.functions:
        for blk in f.blocks:
            il = blk.instructions
            new = []
            changed = False
            for inst in il:
                si = inst.sync_info
                if si is not None and len(si.on_wait) > 1:
                    conds = list(si.on_wait)
                    for c in conds[:-1]:
                        uid += 1
                        nop = mybir.InstNoOp(
                            name=f"splitw-{uid}",
                            engine=inst.engine,
                            sync_info=mybir.SyncInfo(on_wait=[c], on_update=[]),
                            bass_nofuse=True,
                        )
                        new.append(nop)
                    inst.sync_info = _br.SyncInfo(
                        on_wait=[conds[-1]], on_update=list(si.on_update)
                    )
                    changed = True
                new.append(inst)
            if changed:
                blk.instructions = new


def _build_program():
    nc = bass.Bass(num_devices=N_CORES)

    hsT = nc.dram_tensor("hsT", [HID, S], BF16, kind="ExternalInput")
    wqkvT = nc.dram_tensor("wqkvT", [HID, 3 * D], BF16, kind="ExternalInput")
    woT = nc.dram_tensor("woT", [D, HID], BF16, kind="ExternalInput")
    cosT = nc.dram_tensor("cosT", [D, S], BF16, kind="ExternalInput")
    sinT = nc.dram_tensor("sinT", [D, S], BF16, kind="ExternalInput")
    cq = nc.dram_tensor("cq", [D, 1], F32, kind="ExternalInput")
    ck = nc.dram_tensor("ck", [D, 1], F32, kind="ExternalInput")
    mask01 = nc.dram_tensor("mask01", [128, STRIPW], BF16, kind="ExternalInput")
    o_partial = nc.dram_tensor("o_partial", [S, HID], BF16, kind="ExternalOutput")
    # DRAM scratch rows used to broadcast / transpose per-token scalars.
    rq_row_d = nc.dram_tensor("rq_row_d", [1, S], F32)
    rk_row_d = nc.dram_tensor("rk_row_d", [1, S], F32)
    rd_row_d = nc.dram_tensor("rd_row_d", [1, S], F32)

    with TileContext(nc) as tc:
        _emit(nc, tc, hsT, wqkvT, woT, cosT, sinT, cq, ck, mask01,
              o_partial, rq_row_d, rk_row_d, rd_row_d)
    _split_multi_waits(nc)
    return nc


def _emit(nc, tc, hsT, wqkvT, woT, cosT, sinT, cq, ck, mask01,
          o_partial, rq_row_d, rk_row_d, rd_row_d):
    with (
        tc.tile_pool(name="persist", bufs=1) as persist,
        tc.tile_pool(name="rows", bufs=2) as row_pool,
        tc.tile_pool(name="t", bufs=2) as t_pool,
        tc.tile_pool(name="sc_ps", bufs=2, space="PSUM") as sc_ps,
    ):
        # Persistent SBUF tensors (live across phases).
        qn = [persist.tile([128, S], BF16, tag=f"qn{m}", name="t_qn") for m in range(2)]
        kn = [persist.tile([128, S], BF16, tag=f"kn{m}", name="t_kn") for m in range(2)]
        v_sb = persist.tile([128, NSB, D], BF16, tag="v", name="t_v")      # [j, jb, d]
        ao = [persist.tile([128, S], BF16, tag=f"ao{m}", name="t_ao") for m in range(2)]
        wo_sb = persist.tile([128, 2, HID], BF16, tag="wo", name="t_wo")
        mask_sb = persist.tile([128, STRIPW], BF16, tag="mask", name="t_mask")
        cq_sb = persist.tile([128, 2], F32, tag="cq", name="t_cq")
        ck_sb = persist.tile([128, 2], F32, tag="ck", name="t_ck")
        ones_sb = persist.tile([128, 1], BF16, tag="ones", name="t_ones")
        rkT = persist.tile([128, NSB], F32, tag="rkT", name="t_rkT")
        rdT = persist.tile([128, NSB], F32, tag="rdT", name="t_rdT")
        eps_sb = persist.tile([1, 1], F32, tag="eps", name="t_eps")

        nc.gpsimd.dma_start(out=wo_sb,
                            in_=woT.ap().rearrange("(m p) c -> p m c", p=128))
        nc.gpsimd.dma_start(out=mask_sb, in_=mask01.ap())
        nc.gpsimd.dma_start(out=cq_sb,
                            in_=cq.ap().rearrange("(m p) one -> p (m one)", p=128))
        nc.gpsimd.dma_start(out=ck_sb,
                            in_=ck.ap().rearrange("(m p) one -> p (m one)", p=128))
        nc.vector.memset(ones_sb, 1.0)
        nc.vector.memset(eps_sb, EPS)

        tts = [None] * NSB

        def emit_strip_tanh(jb):
            q0 = jb * 128
            w = min(STRIPW, S - q0)
            tt = t_pool.tile([128, STRIPW], BF16, tag="tt", name="t_tt",
                             bufs=9)
            half = (w + 1) // 2
            parts = [(0, half), (half, w - half)] if w > 512 else [(0, w)]
            for off, n in parts:
                ps = sc_ps.tile([128, 512], F32, tag="sc", name="t_sc")
                for m in range(2):
                    nc.tensor.matmul(
                        ps[:, :n], kn[m][:, jb * 128:(jb + 1) * 128],
                        qn[m][:, q0 + off:q0 + off + n],
                        start=(m == 0), stop=(m == 1))
                nc.scalar.activation(tt[:, off:off + n], ps[:, :n],
                                     mybir.ActivationFunctionType.Tanh,
                                     scale=rkT[:, jb:jb + 1])
            tts[jb] = tt

        # ---- Phase B/C: projections + rmsnorm + rope over 4 s-chunks ----
        with (
            tc.tile_pool(name="wpool", bufs=1) as wpool,
            tc.tile_pool(name="hs", bufs=42) as hs_pool,
            tc.tile_pool(name="cs", bufs=2) as cs_pool,
            tc.tile_pool(name="ep", bufs=2) as ep_pool,
            tc.tile_pool(name="qk_ps", bufs=1, space="PSUM") as qk_ps,
            tc.tile_pool(name="v_ps", bufs=2, space="PSUM") as v_ps,
        ):
            wqkv_k = [wpool.tile([128, 3 * D], BF16, tag=f"wqkv{k}",
                                 name="t_wqkv") for k in range(KT)]

            def emit_chunk_b(c, hs_list, off, cos_p, sin_p):
                s0 = c * 512
                qp = [qk_ps.tile([128, 512], F32, tag=f"qp{m}", name="t_qp")
                      for m in range(2)]
                kp = [qk_ps.tile([128, 512], F32, tag=f"kp{m}", name="t_kp")
                      for m in range(2)]
                vp01 = [v_ps.tile([128, D], F32, tag="vp", name="t_vp")
                        for _ in range(2)]
                for k in range(KT):
                    hs_t = hs_list[k]
                    st, sp = (k == 0), (k == KT - 1)
                    for m in range(2):
                        nc.tensor.matmul(qp[m],
                                         wqkv_k[k][:, m * 128:(m + 1) * 128],
                                         hs_t[:, off:off + 512],
                                         start=st, stop=sp)
                    for m in range(2):
                        nc.tensor.matmul(kp[m],
                                         wqkv_k[k][:, D + m * 128:D + (m + 1) * 128],
                                         hs_t[:, off:off + 512],
                                         start=st, stop=sp)
                    for sb in range(2):
                        nc.tensor.matmul(vp01[sb],
                                         hs_t[:, off + sb * 128:off + (sb + 1) * 128],
                                         wqkv_k[k][:, 2 * D:3 * D],
                                         start=st, stop=sp)
                for sb in range(2):
                    nc.vector.tensor_copy(v_sb[:, c * 4 + sb, :], vp01[sb])
                vp23 = [v_ps.tile([128, D], F32, tag="vp", name="t_vp")
                        for _ in range(2)]
                for k in range(KT):
                    st, sp = (k == 0), (k == KT - 1)
                    for sb in (2, 3):
                        nc.tensor.matmul(vp23[sb - 2],
                                         hs_list[k][:, off + sb * 128:off + (sb + 1) * 128],
                                         wqkv_k[k][:, 2 * D:3 * D],
                                         start=st, stop=sp)
                for sb in (2, 3):
                    nc.vector.tensor_copy(v_sb[:, c * 4 + sb, :], vp23[sb - 2])

                # ACT scaled squares (c_d in scale), DVE evacuation copies
                sqs = {}
                for which, pp, cvec in (("q", qp, cq_sb), ("k", kp, ck_sb)):
                    for m in range(2):
                        sq = ep_pool.tile([128, 512], BF16, bufs=1,
                                          tag=f"sq_{which}{m}", name="t_sq")
                        nc.scalar.activation(sq, pp[m],
                                             mybir.ActivationFunctionType.Square,
                                             scale=cvec[:, m:m + 1])
                        sqs[which, m] = sq
                qt = []
                for m in range(2):
                    qt_m = ep_pool.tile([128, 512], BF16, tag=f"qt{m}", name="t_qt")
                    nc.vector.tensor_copy(qt_m, qp[m])
                    qt.append(qt_m)
                kh = []
                for m in range(2):
                    kh_m = ep_pool.tile([128, 512], BF16, tag=f"kh{m}", name="t_kh")
                    nc.vector.tensor_copy(kh_m, kp[m])
                    kh.append(kh_m)

                rows = {}
                for which in ("q", "k"):
                    msq = v_ps.tile([1, 512], F32, tag="vp", name="t_msq")
                    for m in range(2):
                        nc.tensor.matmul(msq, ones_sb, sqs[which, m],
                                         start=(m == 0), stop=(m == 1))
                    rows[which] = msq
                srts = {}
                for which in ("q", "k"):
                    srt = row_pool.tile([1, 512], F32, tag=f"srt_{which}",
                                        name="t_srt")
                    nc.scalar.activation(srt, rows[which],
                                         mybir.ActivationFunctionType.Sqrt,
                                         bias=eps_sb)
                    srts[which] = srt
                rrow_q = row_pool.tile([1, 512], F32, tag="rrow_q", name="t_rrow")
                nc.vector.reciprocal(rrow_q, srts["q"])
                nc.gpsimd.dma_start(out=rq_row_d[0:1, s0:s0 + 512], in_=rrow_q)
                rrow_k = row_pool.tile([1, 512], F32, tag="rrow_k", name="t_rrow")
                nc.vector.reciprocal(rrow_k, srts["k"])
                rrow2 = row_pool.tile([1, 512], F32, tag="rrow2", name="t_rrow2")
                nc.vector.tensor_scalar_mul(rrow2, rrow_k, SCALE / SOFTCAP)
                nc.gpsimd.dma_start(out=rk_row_d[0:1, s0:s0 + 512], in_=rrow2)

                # broadcast rstd_q across partitions; transpose rstd_k to cols
                rq_bc = ep_pool.tile([128, 512], F32, bufs=1, tag="rq_bc", name="t_rq_bc")
                nc.gpsimd.dma_start(
                    out=rq_bc,
                    in_=bass.AP(tensor=rq_row_d, offset=s0,
                                ap=[[0, 128], [1, 512]]))
                nc.gpsimd.dma_start(
                    out=rkT[:, c * 4:(c + 1) * 4],
                    in_=bass.AP(tensor=rk_row_d, offset=s0,
                                ap=[[1, 128], [128, 4]]))

                qh = []
                for m in range(2):
                    qh_m = ep_pool.tile([128, 512], BF16, tag=f"qh{m}", name="t_qh")
                    nc.vector.tensor_mul(qh_m, qt[m], rq_bc)
                    qh.append(qh_m)
                for src_t, dst in ((qh, qn), (kh, kn)):
                    for m in range(2):
                        t0 = ep_pool.tile([128, 512], BF16, tag="rope_t0",
                                          name="t_rope_t0")
                        t1 = ep_pool.tile([128, 512], BF16, tag="rope_t1",
                                          name="t_rope_t1")
                        nc.vector.tensor_mul(t0, src_t[m],
                                             cos_p[m][:, off:off + 512])
                        nc.vector.tensor_mul(t1, src_t[1 - m],
                                             sin_p[m][:, off:off + 512])
                        if m == 0:
                            nc.vector.tensor_sub(dst[m][:, s0:s0 + 512], t0, t1)
                        else:
                            nc.vector.tensor_add(dst[m][:, s0:s0 + 512], t0, t1)

            for c in range(NCH):
                s0 = c * 512
                hs_list = []
                for k in range(KT):
                    hs_t = hs_pool.tile([128, 512], BF16, tag="hs", name="t_hs")
                    nc.sync.dma_start(
                        out=hs_t,
                        in_=hsT[k * 128:(k + 1) * 128, s0:s0 + 512])
                    hs_list.append(hs_t)
                    if c == 0:
                        nc.sync.dma_start(
                            out=wqkv_k[k],
                            in_=wqkvT[k * 128:(k + 1) * 128, :])
                cos_p = [cs_pool.tile([128, 512], BF16, tag=f"cos{m}",
                                      name="t_cos") for m in range(2)]
                sin_p = [cs_pool.tile([128, 512], BF16, tag=f"sin{m}",
                                      name="t_sin") for m in range(2)]
                for m in range(2):
                    nc.gpsimd.dma_start(
                        out=cos_p[m],
                        in_=cosT[m * 128:(m + 1) * 128, s0:s0 + 512])
                    nc.gpsimd.dma_start(
                        out=sin_p[m],
                        in_=sinT[m * 128:(m + 1) * 128, s0:s0 + 512])
                emit_chunk_b(c, hs_list, 0, cos_p, sin_p)

        # ---- Phases D/E/F/G: score strips, softmax, PV, o_proj ----
        with (
            tc.tile_pool(name="e", bufs=9) as e_pool,
            tc.tile_pool(name="oc", bufs=2) as oc_pool,
            tc.tile_pool(name="pv_ps", bufs=3, space="PSUM") as pv_ps,
            tc.tile_pool(name="dn_ps", bufs=1, space="PSUM") as dn_ps,
            tc.tile_pool(name="og_ps", bufs=2, space="PSUM") as og_ps,
        ):
            e_tiles = [None] * NSB

            def emit_strip_exp(jb):
                q0 = jb * 128
                w = min(STRIPW, S - q0)
                er = t_pool.tile([128, STRIPW], BF16, tag="er", name="t_er")
                nc.scalar.activation(er[:, :w], tts[jb][:, :w],
                                     mybir.ActivationFunctionType.Exp,
                                     scale=SOFTCAP)
                e = e_pool.tile([128, STRIPW], BF16, tag="e", name="t_e")
                nc.vector.tensor_mul(e[:, :w], er[:, :w], mask_sb[:, :w])
                e_tiles[jb] = e

            def emit_chunk(c):
                q0 = c * 512
                jbs = [jb for jb in range(max(0, 4 * c - 4), 4 * c + 4)
                       if jb * 128 + STRIPW > q0]
                ops = [pv_ps.tile([128, 512], F32, tag="pv", name="t_pv")
                       for _ in range(2)]
                dn = dn_ps.tile([1, 512], F32, tag="dn", name="t_dn")
                for i, jb in enumerate(jbs):
                    sw = min(STRIPW, S - jb * 128)
                    lo = max(q0, jb * 128)
                    hi = min(q0 + 512, jb * 128 + sw)
                    el = lo - jb * 128
                    co = lo - q0
                    n = hi - lo
                    st, sp = (i == 0), (i == len(jbs) - 1)
                    e = e_tiles[jb]
                    for m in range(2):
                        nc.tensor.matmul(ops[m][:, co:co + n],
                                         v_sb[:, jb, m * 128:(m + 1) * 128],
                                         e[:, el:el + n], start=st, stop=sp,
                                         skip_group_check=True)
                    nc.tensor.matmul(dn[:, co:co + n], ones_sb,
                                     e[:, el:el + n], start=st, stop=sp,
                                     skip_group_check=True)
                rd = row_pool.tile([1, 512], F32, tag="rd", name="t_rd")
                nc.vector.reciprocal(rd, dn)
                nc.sync.dma_start(out=rd_row_d[0:1, q0:q0 + 512], in_=rd)
                nc.sync.dma_start(
                    out=rdT[:, c * 4:(c + 1) * 4],
                    in_=bass.AP(tensor=rd_row_d, offset=q0,
                                ap=[[1, 128], [128, 4]]))
                for m in range(2):
                    nc.vector.tensor_copy(ao[m][:, q0:q0 + 512], ops[m])

            def emit_oproj(sb):
                oc = oc_pool.tile([128, HID], BF16, tag="oc", name="t_oc")
                for cc in range(HID // 512):
                    op = og_ps.tile([128, 512], F32, tag="og", name="t_og")
                    for m in range(2):
                        nc.tensor.matmul(op,
                                         ao[m][:, sb * 128:(sb + 1) * 128],
                                         wo_sb[:, m, cc * 512:(cc + 1) * 512],
                                         start=(m == 0), stop=(m == 1))
                    if cc % 2 == 0:
                        nc.vector.tensor_scalar_mul(
                            oc[:, cc * 512:(cc + 1) * 512], op,
                            rdT[:, sb:sb + 1])
                    else:
                        nc.scalar.activation(
                            oc[:, cc * 512:(cc + 1) * 512], op,
                            mybir.ActivationFunctionType.Copy,
                            scale=rdT[:, sb:sb + 1])
                nc.sync.dma_start(
                    out=o_partial[sb * 128:(sb + 1) * 128, :], in_=oc)

            # Interleave strips -> chunk PV -> this chunk's o_proj.
            for c in range(NCH):
                for jb in range(4 * c, 4 * c + 4):
                    emit_strip_tanh(jb)
                for jb in range(4 * c, 4 * c + 4):
                    emit_strip_exp(jb)
                emit_chunk(c)
                for sb in range(4 * c, 4 * c + 4):
                    emit_oproj(sb)


_PROGRAM = None


def _get_program():
    global _PROGRAM
    if _PROGRAM is None:
        _PROGRAM = _build_program()
    return _PROGRAM


def _prep_inputs(hidden_states, position_ids, cos_table, sin_table,
                 Wq, Wk, Wv, Wo, q_norm_w, k_norm_w):
    bf16 = ml_dtypes.bfloat16
    f32 = np.float32
    hs = np.asarray(hidden_states, dtype=f32).reshape(S, HID)
    hsT = np.ascontiguousarray(hs.T).astype(bf16)
    pos = np.asarray(position_ids).reshape(S).astype(np.int64)
    cosT = np.ascontiguousarray(
        np.asarray(cos_table, dtype=f32)[pos].T).astype(bf16)
    sinT = np.ascontiguousarray(
        np.asarray(sin_table, dtype=f32)[pos].T).astype(bf16)
    Wq = np.asarray(Wq, dtype=f32); Wk = np.asarray(Wk, dtype=f32)
    Wv = np.asarray(Wv, dtype=f32); Wo = np.asarray(Wo, dtype=f32)
    qw = 1.0 + np.asarray(q_norm_w, dtype=f32)
    kw = 1.0 + np.asarray(k_norm_w, dtype=f32)
    cq = (np.abs(qw) ** -1 / np.sqrt(D)).astype(f32).reshape(D, 1)
    ck = (np.abs(kw) ** -1 / np.sqrt(D)).astype(f32).reshape(D, 1)
    p = np.arange(128)[:, None]
    f = np.arange(STRIPW)[None, :]
    mask01 = (((f - p) >= 0) & ((f - p) < WINDOW)).astype(bf16)

    in_maps = []
    for h in range(N_CORES):
        kv = h // (H // KV)
        wq_s = Wq[h * D:(h + 1) * D, :] * qw[:, None]
        wk_s = Wk[kv * D:(kv + 1) * D, :] * kw[:, None]
        wv_s = Wv[kv * D:(kv + 1) * D, :]
        wo_s = Wo[:, h * D:(h + 1) * D]
        wqkv = np.concatenate([wq_s.T, wk_s.T, wv_s.T], axis=1)
        in_maps.append({
            "hsT": hsT,
            "wqkvT": np.ascontiguousarray(wqkv).astype(bf16),
            "woT": np.ascontiguousarray(wo_s.T).astype(bf16),
            "cosT": cosT, "sinT": sinT,
            "cq": cq, "ck": ck, "mask01": mask01,
        })
    return in_maps


def _run(in_maps, trace=False):
    nc = _get_program()
    return run_bass_kernel_spmd(nc, in_maps, list(range(N_CORES)), trace=trace)


def kernel(hidden_states, position_ids, cos_table, sin_table,
           Wq, Wk, Wv, Wo, q_norm_w, k_norm_w):
    in_maps = _prep_inputs(hidden_states, position_ids, cos_table, sin_table,
                           Wq, Wk, Wv, Wo, q_norm_w, k_norm_w)
    res = _run(in_maps)
    out = np.zeros((S, HID), dtype=np.float32)
    for r in res.results:
        out += r["o_partial"].astype(np.float32)
    return out.reshape(B, S, HID)


# revision 35
# speedup vs baseline: 1.1666x; 1.0325x over previous
"""Gemma3 sliding-window attention on 8 Trainium2 NeuronCores.

Sharding: tensor-parallel over the 8 query heads (1 head per core). Each core
computes q/k/v projections for its head (KV head replicated per pair of
cores), RMS norm + RoPE, windowed-causal softcapped attention, and its o_proj
partial; the host sums the 8 partials.

Kernel layout notes:
- All matmuls run in bf16 (fp32 PSUM accumulation).
- Scores are computed transposed (sT[j, q] strips, j on partitions) so the
  exp output is directly the PV matmul's moving-side input - no PE
  transposes anywhere.
- Softcap tanh bounds logits to +-SOFTCAP, so softmax runs without the
  max-subtraction pass (exp(+-50) is safe in fp32).
- (1 + rms_w) is folded into Wq/Wk on the host; rstd_q is applied to q via a
  DRAM-broadcast row; rstd_k is folded into the tanh activation scale;
  1/denominator is folded into the o_proj PSUM-evacuation scale.
"""

import numpy as np
import ml_dtypes

import concourse.bass as bass
from concourse import mybir
from concourse.bass_utils import run_bass_kernel_spmd
from concourse.tile import TileContext

B, S, HID = 1, 2048, 2560
H, KV, D = 8, 4, 256
SCALE = 256 ** -0.5
SOFTCAP = 50.0
WINDOW = 512
EPS = 1e-6

N_CORES = 8
BF16 = mybir.dt.bfloat16
F32 = mybir.dt.float32
KT = HID // 128          # 20 contraction tiles for projections
NCH = S // 512           # 4 query chunks of 512
NSB = S // 128           # 16 seq blocks of 128
STRIPW = WINDOW + 128    # 640: score strip width per j-block
SH = S // 2              # per-core K/V sequence half
PAIRS = [[0, 1], [2, 3], [4, 5], [6, 7]]


def _split_multi_waits(nc):
    """This walrus build accepts only ONE sync-wait per instruction, but Tile
    attaches one wait per cross-proc dependency. Hoist extra waits onto
    freshly inserted same-engine NOPs directly before the instruction."""
    import bass_rust as _br

    uid = 0
    for f in nc.m.functions:
        for blk in f.blocks:
            il = blk.instructions
            new = []
            changed = False
            for inst in il:
                si = inst.sync_info
                if si is not None and len(si.on_wait) > 1:
                    conds = list(si.on_wait)
                    for c in conds[:-1]:
                        uid += 1
                        nop = mybir.InstNoOp(
                            name=f"splitw-{uid}",
                            engine=inst.engine,
                            sync_info=mybir.SyncInfo(on_wait=[c], on_update=[]),
                            bass_nofuse=True,
                        )
                        new.append(nop)
                    inst.sync_info = _br.SyncInfo(
                        on_wait=[conds[-1]], on_update=list(si.on_update)
                    )
                    changed = True
                new.append(inst)
            if changed:
                blk.instructions = new


def _build_program():
    nc = bass.Bass(num_devices=N_CORES)

    hsT = nc.dram_tensor("hsT", [HID, S], BF16, kind="ExternalInput")
    wqkvT = nc.dram_tensor("wqkvT", [HID, 3 * D], BF16, kind="ExternalInput")
    woT = nc.dram_tensor("woT", [D, HID], BF16, kind="ExternalInput")
    cosT = nc.dram_tensor("cosT", [D, S], BF16, kind="ExternalInput")
    sinT = nc.dram_tensor("sinT", [D, S], BF16, kind="ExternalInput")
    cq = nc.dram_tensor("cq", [D, 1], F32, kind="ExternalInput")
    ck = nc.dram_tensor("ck", [D, 1], F32, kind="ExternalInput")
    mask01 = nc.dram_tensor("mask01", [128, STRIPW], BF16, kind="ExternalInput")
    o_partial = nc.dram_tensor("o_partial", [S, HID], BF16, kind="ExternalOutput")
    # DRAM scratch rows used to broadcast / transpose per-token scalars.
    rq_row_d = nc.dram_tensor("rq_row_d", [1, S], F32)
    rk_row_d = nc.dram_tensor("rk_row_d", [1, S], F32)
    rd_row_d = nc.dram_tensor("rd_row_d", [1, S], F32)

    with TileContext(nc) as tc:
        _emit(nc, tc, hsT, wqkvT, woT, cosT, sinT, cq, ck, mask01,
              o_partial, rq_row_d, rk_row_d, rd_row_d)
    _split_multi_waits(nc)
    return nc


def _emit(nc, tc, hsT, wqkvT, woT, cosT, sinT, cq, ck, mask01,
          o_partial, rq_row_d, rk_row_d, rd_row_d):
    with (
        tc.tile_pool(name="persist", bufs=1) as persist,
        tc.tile_pool(name="rows", bufs=2) as row_pool,
        tc.tile_pool(name="t", bufs=2) as t_pool,
        tc.tile_pool(name="sc_ps", bufs=2, space="PSUM") as sc_ps,
    ):
        # Persistent SBUF tensors (live across phases).
        qn = [persist.tile([128, S], BF16, tag=f"qn{m}", name="t_qn") for m in range(2)]
        kn = [persist.tile([128, S], BF16, tag=f"kn{m}", name="t_kn") for m in range(2)]
        v_sb = persist.tile([128, NSB, D], BF16, tag="v", name="t_v")      # [j, jb, d]
        ao = [persist.tile([128, S], BF16, tag=f"ao{m}", name="t_ao") for m in range(2)]
        wo_sb = persist.tile([128, 2, HID], BF16, tag="wo", name="t_wo")
        mask_sb = persist.tile([128, STRIPW], BF16, tag="mask", name="t_mask")
        cq_sb = persist.tile([128, 2], F32, tag="cq", name="t_cq")
        ck_sb = persist.tile([128, 2], F32, tag="ck", name="t_ck")
        ones_sb = persist.tile([128, 1], BF16, tag="ones", name="t_ones")
        rkT = persist.tile([128, NSB], F32, tag="rkT", name="t_rkT")
        rdT = persist.tile([128, NSB], F32, tag="rdT", name="t_rdT")
        eps_sb = persist.tile([1, 1], F32, tag="eps", name="t_eps")

        nc.gpsimd.dma_start(out=wo_sb,
                            in_=woT.ap().rearrange("(m p) c -> p m c", p=128))
        nc.gpsimd.dma_start(out=mask_sb, in_=mask01.ap())
        nc.gpsimd.dma_start(out=cq_sb,
                            in_=cq.ap().rearrange("(m p) one -> p (m one)", p=128))
        nc.gpsimd.dma_start(out=ck_sb,
                            in_=ck.ap().rearrange("(m p) one -> p (m one)", p=128))
        nc.vector.memset(ones_sb, 1.0)
        nc.vector.memset(eps_sb, EPS)

        tts = [None] * NSB

        def emit_strip_tanh(jb):
            q0 = jb * 128
            w = min(STRIPW, S - q0)
            tt = t_pool.tile([128, STRIPW], BF16, tag="tt", name="t_tt",
                             bufs=9)
            half = (w + 1) // 2
            parts = [(0, half), (half, w - half)] if w > 512 else [(0, w)]
            for off, n in parts:
                ps = sc_ps.tile([128, 512], F32, tag="sc", name="t_sc")
                for m in range(2):
                    nc.tensor.matmul(
                        ps[:, :n], kn[m][:, jb * 128:(jb + 1) * 128],
                        qn[m][:, q0 + off:q0 + off + n],
                        start=(m == 0), stop=(m == 1))
                nc.scalar.activation(tt[:, off:off + n], ps[:, :n],
                                     mybir.ActivationFunctionType.Tanh,
                                     scale=rkT[:, jb:jb + 1])
            tts[jb] = tt

        # ---- Phase B/C: projections + rmsnorm + rope over 4 s-chunks ----
        with (
            tc.tile_pool(name="wpool", bufs=1) as wpool,
            tc.tile_pool(name="hs", bufs=42) as hs_pool,
            tc.tile_pool(name="cs", bufs=2) as cs_pool,
            tc.tile_pool(name="ep", bufs=2) as ep_pool,
            tc.tile_pool(name="qk_ps", bufs=1, space="PSUM") as qk_ps,
            tc.tile_pool(name="v_ps", bufs=2, space="PSUM") as v_ps,
        ):
            wqkv_k = [wpool.tile([128, 3 * D], BF16, tag=f"wqkv{k}",
                                 name="t_wqkv") for k in range(KT)]

            def emit_chunk_b(c, hs_list, off, cos_p, sin_p):
                s0 = c * 512
                qp = [qk_ps.tile([128, 512], F32, tag=f"qp{m}", name="t_qp")
                      for m in range(2)]
                kp = [qk_ps.tile([128, 512], F32, tag=f"kp{m}", name="t_kp")
                      for m in range(2)]
                vp01 = [v_ps.tile([128, D], F32, tag="vp", name="t_vp")
                        for _ in range(2)]
                for k in range(KT):
                    hs_t = hs_list[k]
                    st, sp = (k == 0), (k == KT - 1)
                    for m in range(2):
                        nc.tensor.matmul(qp[m],
                                         wqkv_k[k][:, m * 128:(m + 1) * 128],
                                         hs_t[:, off:off + 512],
                                         start=st, stop=sp)
                    for m in range(2):
                        nc.tensor.matmul(kp[m],
                                         wqkv_k[k][:, D + m * 128:D + (m + 1) * 128],
                                         hs_t[:, off:off + 512],
                                         start=st, stop=sp)
                    for sb in range(2):
                        nc.tensor.matmul(vp01[sb],
                                         hs_t[:, off + sb * 128:off + (sb + 1) * 128],
                                         wqkv_k[k][:, 2 * D:3 * D],
                                         start=st, stop=sp)
                for sb in range(2):
                    nc.vector.tensor_copy(v_sb[:, c * 4 + sb, :], vp01[sb])
                vp23 = [v_ps.tile([128, D], F32, tag="vp", name="t_vp")
                        for _ in range(2)]
                for k in range(KT):
                    st, sp = (k == 0), (k == KT - 1)
                    for sb in (2, 3):
                        nc.tensor.matmul(vp23[sb - 2],
                                         hs_list[k][:, off + sb * 128:off + (sb + 1) * 128],
                                         wqkv_k[k][:, 2 * D:3 * D],
                                         start=st, stop=sp)
                for sb in (2, 3):
                    nc.vector.tensor_copy(v_sb[:, c * 4 + sb, :], vp23[sb - 2])

                # ACT scaled squares (c_d in scale), DVE evacuation copies
                sqs = {}
                for which, pp, cvec in (("q", qp, cq_sb), ("k", kp, ck_sb)):
                    for m in range(2):
                        sq = ep_pool.tile([128, 512], BF16, bufs=1,
                                          tag=f"sq_{which}{m}", name="t_sq")
                        nc.scalar.activation(sq, pp[m],
                                             mybir.ActivationFunctionType.Square,
                                             scale=cvec[:, m:m + 1])
                        sqs[which, m] = sq
                qt = []
                for m in range(2):
                    qt_m = ep_pool.tile([128, 512], BF16, tag=f"qt{m}", name="t_qt")
                    nc.vector.tensor_copy(qt_m, qp[m])
                    qt.append(qt_m)
                kh = []
                for m in range(2):
                    kh_m = ep_pool.tile([128, 512], BF16, tag=f"kh{m}", name="t_kh")
                    nc.vector.tensor_copy(kh_m, kp[m])
                    kh.append(kh_m)

                rows = {}
                for which in ("q", "k"):
                    msq = v_ps.tile([1, 512], F32, tag="vp", name="t_msq")
                    for m in range(2):
                        nc.tensor.matmul(msq, ones_sb, sqs[which, m],
                                         start=(m == 0), stop=(m == 1))
                    rows[which] = msq
                srts = {}
                for which in ("q", "k"):
                    srt = row_pool.tile([1, 512], F32, tag=f"srt_{which}",
                                        name="t_srt")
                    nc.scalar.activation(srt, rows[which],
                                         mybir.ActivationFunctionType.Sqrt,
                                         bias=eps_sb)
                    srts[which] = srt
                rrow_q = row_pool.tile([1, 512], F32, tag="rrow_q", name="t_rrow")
                nc.vector.reciprocal(rrow_q, srts["q"])
                nc.gpsimd.dma_start(out=rq_row_d[0:1, s0:s0 + 512], in_=rrow_q)
                rrow_k = row_pool.tile([1, 512], F32, tag="rrow_k", name="t_rrow")
                nc.vector.reciprocal(rrow_k, srts["k"])
                rrow2 = row_pool.tile([1, 512], F32, tag="rrow2", name="t_rrow2")
                nc.vector.tensor_scalar_mul(rrow2, rrow_k, SCALE / SOFTCAP)
                nc.gpsimd.dma_start(out=rk_row_d[0:1, s0:s0 + 512], in_=rrow2)

                # broadcast rstd_q across partitions; transpose rstd_k to cols
                rq_bc = ep_pool.tile([128, 512], F32, bufs=1, tag="rq_bc", name="t_rq_bc")
                nc.gpsimd.dma_start(
                    out=rq_bc,
                    in_=bass.AP(tensor=rq_row_d, offset=s0,
                                ap=[[0, 128], [1, 512]]))
                nc.gpsimd.dma_start(
                    out=rkT[:, c * 4:(c + 1) * 4],
                    in_=bass.AP(tensor=rk_row_d, offset=s0,
                                ap=[[1, 128], [128, 4]]))

                qh = []
                for m in range(2):
                    qh_m = ep_pool.tile([128, 512], BF16, tag=f"qh{m}", name="t_qh")
                    nc.vector.tensor_mul(qh_m, qt[m], rq_bc)
                    qh.append(qh_m)
                for src_t, dst in ((qh, qn), (kh, kn)):
                    for m in range(2):
                        t0 = ep_pool.tile([128, 512], BF16, tag="rope_t0",
                                          name="t_rope_t0")
                        t1 = ep_pool.tile([128, 512], BF16, tag="rope_t1",
                                          name="t_rope_t1")
                        nc.vector.tensor_mul(t0, src_t[m],
                                             cos_p[m][:, off:off + 512])
                        nc.vector.tensor_mul(t1, src_t[1 - m],
                                             sin_p[m][:, off:off + 512])
                        if m == 0:
                            nc.vector.tensor_sub(dst[m][:, s0:s0 + 512], t0, t1)
                        else:
                            nc.vector.tensor_add(dst[m][:, s0:s0 + 512], t0, t1)

            for c in range(NCH):
                s0 = c * 512
                hs_list = []
                for k in range(KT):
                    hs_t = hs_pool.tile([128, 512], BF16, tag="hs", name="t_hs")
                    nc.sync.dma_start(
                        out=hs_t,
                        in_=hsT[k * 128:(k + 1) * 128, s0:s0 + 512])
                    hs_list.append(hs_t)
                    if c == 0:
                        nc.sync.dma_start(
                            out=wqkv_k[k],
                            in_=wqkvT[k * 128:(k + 1) * 128, :])
                cos_p = [cs_pool.tile([128, 512], BF16, tag=f"cos{m}",
                                      name="t_cos") for m in range(2)]
                sin_p = [cs_pool.tile([128, 512], BF16, tag=f"sin{m}",
                                      name="t_sin") for m in range(2)]
                for m in range(2):
                    nc.gpsimd.dma_start(
                        out=cos_p[m],
                        in_=cosT[m * 128:(m + 1) * 128, s0:s0 + 512])
                    nc.gpsimd.dma_start(
                        out=sin_p[m],
                        in_=sinT[m * 128:(m + 1) * 128, s0:s0 + 512])
                emit_chunk_b(c, hs_list, 0, cos_p, sin_p)

        # ---- Phases D/E/F/G: score strips, softmax, PV, o_proj ----
        with (
            tc.tile_pool(name="e", bufs=9) as e_pool,
            tc.tile_pool(name="oc", bufs=2) as oc_pool,
            tc.tile_pool(name="pv_ps", bufs=2, space="PSUM") as pv_ps,
            tc.tile_pool(name="dn_ps", bufs=1, space="PSUM") as dn_ps,
            tc.tile_pool(name="og_ps", bufs=3, space="PSUM") as og_ps,
        ):
            e_tiles = [None] * NSB

            def emit_strip_exp(jb):
                q0 = jb * 128
                w = min(STRIPW, S - q0)
                er = t_pool.tile([128, STRIPW], BF16, tag="er", name="t_er", bufs=4)
                nc.scalar.activation(er[:, :w], tts[jb][:, :w],
                                     mybir.ActivationFunctionType.Exp,
                                     scale=SOFTCAP)
                e = e_pool.tile([128, STRIPW], BF16, tag="e", name="t_e")
                nc.vector.tensor_mul(e[:, :w], er[:, :w], mask_sb[:, :w])
                e_tiles[jb] = e

            def emit_chunk(c):
                q0 = c * 512
                jbs = [jb for jb in range(max(0, 4 * c - 4), 4 * c + 4)
                       if jb * 128 + STRIPW > q0]
                ops = [pv_ps.tile([128, 512], F32, tag="pv", name="t_pv")
                       for _ in range(2)]
                dn = dn_ps.tile([1, 512], F32, tag="dn", name="t_dn")
                for i, jb in enumerate(jbs):
                    sw = min(STRIPW, S - jb * 128)
                    lo = max(q0, jb * 128)
                    hi = min(q0 + 512, jb * 128 + sw)
                    el = lo - jb * 128
                    co = lo - q0
                    n = hi - lo
                    st, sp = (i == 0), (i == len(jbs) - 1)
                    e = e_tiles[jb]
                    for m in range(2):
                        nc.tensor.matmul(ops[m][:, co:co + n],
                                         v_sb[:, jb, m * 128:(m + 1) * 128],
                                         e[:, el:el + n], start=st, stop=sp,
                                         skip_group_check=True)
                    nc.tensor.matmul(dn[:, co:co + n], ones_sb,
                                     e[:, el:el + n], start=st, stop=sp,
                                     skip_group_check=True)
                rd = row_pool.tile([1, 512], F32, tag="rd", name="t_rd")
                nc.vector.reciprocal(rd, dn)
                nc.sync.dma_start(out=rd_row_d[0:1, q0:q0 + 512], in_=rd)
                nc.sync.dma_start(
                    out=rdT[:, c * 4:(c + 1) * 4],
                    in_=bass.AP(tensor=rd_row_d, offset=q0,
                                ap=[[1, 128], [128, 4]]))
                for m in range(2):
                    nc.vector.tensor_copy(ao[m][:, q0:q0 + 512], ops[m])

            def emit_oproj(sb):
                oc = oc_pool.tile([128, HID], BF16, tag="oc", name="t_oc")
                for cc in range(HID // 512):
                    op = og_ps.tile([128, 512], F32, tag="og", name="t_og")
                    for m in range(2):
                        nc.tensor.matmul(op,
                                         ao[m][:, sb * 128:(sb + 1) * 128],
                                         wo_sb[:, m, cc * 512:(cc + 1) * 512],
                                         start=(m == 0), stop=(m == 1))
                    if cc % 2 == 0:
                        nc.vector.tensor_scalar_mul(
                            oc[:, cc * 512:(cc + 1) * 512], op,
                            rdT[:, sb:sb + 1])
                    else:
                        nc.scalar.activation(
                            oc[:, cc * 512:(cc + 1) * 512], op,
                            mybir.ActivationFunctionType.Copy,
                            scale=rdT[:, sb:sb + 1])
                nc.sync.dma_start(
                    out=o_partial[sb * 128:(sb + 1) * 128, :], in_=oc)

            # Interleave strips -> chunk PV -> this chunk's o_proj.
            for c in range(NCH):
                for jb in range(4 * c, 4 * c + 4):
                    emit_strip_tanh(jb)
                for jb in range(4 * c, 4 * c + 4):
                    emit_strip_exp(jb)
                emit_chunk(c)
                for sb in range(4 * c, 4 * c + 4):
                    emit_oproj(sb)


_PROGRAM = None


def _get_program():
    global _PROGRAM
    if _PROGRAM is None:
        _PROGRAM = _build_program()
    return _PROGRAM


def _prep_inputs(hidden_states, position_ids, cos_table, sin_table,
                 Wq, Wk, Wv, Wo, q_norm_w, k_norm_w):
    bf16 = ml_dtypes.bfloat16
    f32 = np.float32
    hs = np.asarray(hidden_states, dtype=f32).reshape(S, HID)
    hsT = np.ascontiguousarray(hs.T).astype(bf16)
    pos = np.asarray(position_ids).reshape(S).astype(np.int64)
    cosT = np.ascontiguousarray(
        np.asarray(cos_table, dtype=f32)[pos].T).astype(bf16)
    sinT = np.ascontiguousarray(
        np.asarray(sin_table, dtype=f32)[pos].T).astype(bf16)
    Wq = np.asarray(Wq, dtype=f32); Wk = np.asarray(Wk, dtype=f32)
    Wv = np.asarray(Wv, dtype=f32); Wo = np.asarray(Wo, dtype=f32)
    qw = 1.0 + np.asarray(q_norm_w, dtype=f32)
    kw = 1.0 + np.asarray(k_norm_w, dtype=f32)
    cq = (np.abs(qw) ** -1 / np.sqrt(D)).astype(f32).reshape(D, 1)
    ck = (np.abs(kw) ** -1 / np.sqrt(D)).astype(f32).reshape(D, 1)
    p = np.arange(128)[:, None]
    f = np.arange(STRIPW)[None, :]
    mask01 = (((f - p) >= 0) & ((f - p) < WINDOW)).astype(bf16)

    in_maps = []
    for h in range(N_CORES):
        kv = h // (H // KV)
        wq_s = Wq[h * D:(h + 1) * D, :] * qw[:, None]
        wk_s = Wk[kv * D:(kv + 1) * D, :] * kw[:, None]
        wv_s = Wv[kv * D:(kv + 1) * D, :]
        wo_s = Wo[:, h * D:(h + 1) * D]
        wqkv = np.concatenate([wq_s.T, wk_s.T, wv_s.T], axis=1)
        in_maps.append({
            "hsT": hsT,
            "wqkvT": np.ascontiguousarray(wqkv).astype(bf16),
            "woT": np.ascontiguousarray(wo_s.T).astype(bf16),
            "cosT": cosT, "sinT": sinT,
            "cq": cq, "ck": ck, "mask01": mask01,
        })
    return in_maps


def _run(in_maps, trace=False):
    nc = _get_program()
    return run_bass_kernel_spmd(nc, in_maps, list(range(N_CORES)), trace=trace)


def kernel(hidden_states, position_ids, cos_table, sin_table,
           Wq, Wk, Wv, Wo, q_norm_w, k_norm_w):
    in_maps = _prep_inputs(hidden_states, position_ids, cos_table, sin_table,
                           Wq, Wk, Wv, Wo, q_norm_w, k_norm_w)
    res = _run(in_maps)
    out = np.zeros((S, HID), dtype=np.float32)
    for r in res.results:
        out += r["o_partial"].astype(np.float32)
    return out.reshape(B, S, HID)
